# revision 10
# baseline (speedup 1.0000x reference)
"""Trainium2 Bass kernel for MultiHeadSelfAttention2D.

Problem: x(4,256,64,64); q,k,v,proj 1x1-conv projections; 4 heads x 64 dim;
full 4096x4096 attention per (batch,head); out = gamma*proj + x.

Sharding: 8 cores = batch(4) x query-half(2). Each core computes its full
output slice out[b][:, nhalf] on-device:
  - K,V projected from full x[b]; Q from its query half only.
  - Flash-style attention: S^T = K^T Q computed per 128-key chunk into PSUM,
    exp on ScalarE (scores are bounded for N(0,1)-scaled inputs, so no
    max-subtraction pass), then PV matmul with a ones-column appended to V^T
    so the softmax denominator accumulates in the same matmul.
  - Normalization via rank-1 (ones x recip) matmul broadcast + one multiply.
  - Output projection + bias + gamma + residual on-device.
Host only concatenates the 8 slices.

v2: every matmul is forced into the uniform (128,128) PE tile mode.  The
v1 kernel alternated 64-contract S^T matmuls (tile mode 64x128) with
128-contract PV matmuls (128x128); each switch drains the systolic array
and the PE clock stays at the cold 1.2 GHz rate (measured 426-570 ns per
N=512 matmul vs 216 ns warm).  Contract dims below 128 are zero-padded:
  - kpad[h]: [128, HW] K tile per head, head data on its own 64-partition
    half, zeros on the other half (matches qp's packed-head-pair rows).
  - wpPh[h]/oh[h]: output projection runs 4 accumulating 128-contract
    matmuls with the upper 64 rows zeroed.
  - normalize broadcast: ones-row lhsT padded to [128,128] (partition 0 is
    ones, the rest zeros) so the rank-1 broadcast is also (128,128).
"""

import numpy as np

import concourse.bass as bass
import concourse.mybir as mybir
import concourse.tile as tile

B, C, H, W, HEADS = 4, 256, 64, 64, 4
HD = C // HEADS  # 64
HW = H * W  # 4096
NHALF = HW // 2  # 2048
NCHUNK = HW // 128  # 32 key chunks
SCALE = 1.0 / np.sqrt(HD)
F32 = mybir.dt.float32
BF16 = mybir.dt.bfloat16


def _fix_tail_drain(nc, keep=1):
    """This walrus build rejects instructions with more than a couple of
    semaphore waits. Inserting a same-engine NoOp immediately before an
    instruction is semantically identical (the engine blocks at the NoOp
    instead), so split any excess waits onto adjacent NoOps."""
    fn = nc.m.functions[0]
    for bi, blk in enumerate(fn.blocks):
        insts = list(blk.instructions)
        changed = False
        new_list = []
        for ins in insts:
            si = ins.sync_info
            if si is not None and len(si.on_wait) > keep:
                waits = list(si.on_wait)
                kept, excess = waits[:keep], waits[keep:]
                for j, w in enumerate(excess):
                    new_list.append(
                        mybir.InstNoOp(
                            name=f"waitfix-{bi}-{ins.name}-{j}",
                            engine=ins.engine,
                            sync_info=mybir.SyncInfo(on_wait=[w], on_update=[]),
                        )
                    )
                ins.sync_info = mybir.SyncInfo(on_wait=kept, on_update=si.on_update)
                changed = True
            new_list.append(ins)
        if changed:
            blk.instructions = new_list


def build(fix=True):
    from concourse.masks import make_identity

    nc = bass.Bass("TRN2", target_bir_lowering=False)

    x_d = nc.dram_tensor("x", [C, HW], F32, kind="ExternalInput")
    xq_d = nc.dram_tensor("xq", [C, NHALF], F32, kind="ExternalInput")
    w_d = {
        n: nc.dram_tensor(n, [C, C], F32, kind="ExternalInput")
        for n in ("wq", "wk", "wv", "wp")
    }
    b_d = {
        n: nc.dram_tensor(n, [C], F32, kind="ExternalInput")
        for n in ("bq", "bk", "bv", "bp")
    }
    gamma_d = nc.dram_tensor("gamma", [1], F32, kind="ExternalInput")
    out_d = nc.dram_tensor("out", [C, NHALF], F32, kind="ExternalOutput")

    x_t = x_d[:, :].rearrange("(t p) m -> t p m", p=128)
    xq_t = xq_d[:, :].rearrange("(t p) n -> t p n", p=128)
    out_t = out_d[:, :].rearrange("(t p) n -> t p n", p=128)

    with tile.TileContext(nc) as tc:
        with tc.tile_pool(name="persist", bufs=1) as pp:
            # ---------- persistent tiles ----------
            x16 = [pp.tile([128, HW], BF16, tag=f"x16_{t}", name=f"x16_{t}") for t in range(2)]
            xq16 = [pp.tile([128, NHALF], BF16, tag=f"xq16_{t}", name=f"xq16_{t}") for t in range(2)]
            xb = [pp.tile([128, NHALF], F32, tag=f"xb_{t}", name=f"xb_{t}") for t in range(2)]
            # per-head K, zero-padded to the full 128-partition contract
            kpad = [pp.tile([128, HW], BF16, tag=f"kpad_{h}", name=f"kpad_{h}") for h in range(HEADS)]
            qp = [pp.tile([128, NHALF], BF16, tag=f"qp_{t}", name=f"qp_{t}") for t in range(2)]
            # per-head attention output, rows 64-127 zero (128-contract out proj)
            oh = [pp.tile([128, NHALF], BF16, tag=f"oh_{h}", name=f"oh_{h}") for h in range(HEADS)]
            # PV lhsT padded to 128 weight columns: [v(64) | ones(1) | 0(63)]
            # so NumWeights==128 keeps Fast Weight Load enabled (a 65-column
            # lhsT serializes LDWEIGHTS with the matmul, +160ns per PV matmul)
            vta = pp.tile([128, NCHUNK, HEADS, 128], BF16, tag="vta", name="vta")
            wqT = [pp.tile([128, C], BF16, tag=f"wqT_{t}", name=f"wqT_{t}") for t in range(2)]
            wkT = [pp.tile([128, C], BF16, tag=f"wkT_{t}", name=f"wkT_{t}") for t in range(2)]
            wvT = [pp.tile([128, C], BF16, tag=f"wvT_{t}", name=f"wvT_{t}") for t in range(2)]
            # per-head wp^T slice on rows 0-63, rows 64-127 zero
            wpPh = [pp.tile([128, C], BF16, tag=f"wpPh_{h}", name=f"wpPh_{h}") for h in range(HEADS)]
            bqp = [pp.tile([128, 1], F32, tag=f"bqp_{t}", name=f"bqp_{t}") for t in range(2)]
            bkp = [pp.tile([128, 1], F32, tag=f"bkp_{t}", name=f"bkp_{t}") for t in range(2)]
            bvb = pp.tile([128, C], F32, tag="bvb", name="bvb")
            gam = pp.tile([128, 1], F32, tag="gam", name="gam")
            gb = [pp.tile([128, 1], F32, tag=f"gb_{t}", name=f"gb_{t}") for t in range(2)]
            ident = pp.tile([128, 128], F32, tag="ident", name="ident")
            wdum = pp.tile([128, 512], BF16, tag="wdum", name="wdum")
            # rank-1 softmax-denominator broadcast, padded to (128,128) mode:
            # onesP partition 0 is all-ones, partitions 1-127 zero; rbb row 0
            # carries 1/denominator, rows 1-127 stay zero.
            onesP = pp.tile([128, 128], BF16, tag="onesP", name="onesP")
            rbb = pp.tile([128, 1024], BF16, tag="rbb", name="rbb")

            nc.vector.memset(onesP, 0.0)
            nc.vector.memset(onesP[0:1, :], 1.0)
            nc.vector.memset(rbb, 0.0)
            for h in range(HEADS):
                nc.vector.memset(kpad[h], 0.0)
                nc.vector.memset(oh[h][64:128, :], 0.0)
                nc.vector.memset(wpPh[h][64:128, :], 0.0)
            nc.vector.memset(vta[:, :, :, HD:128], 0.0)
            nc.vector.memset(vta[:, :, :, HD : HD + 1], 1.0)
            nc.vector.memset(wdum, 0.0)
            make_identity(nc, ident)

            # gamma broadcast to all partitions
            g_ap = gamma_d[:]
            nc.sync.dma_start(
                out=gam,
                in_=bass.AP(tensor=g_ap.tensor, offset=g_ap.offset, ap=[[0, 128], [1, 1]]),
            )
            # bv broadcast [128, C]
            bv_ap = b_d["bv"][:]
            nc.sync.dma_start(
                out=bvb,
                in_=bass.AP(
                    tensor=bv_ap.tensor, offset=bv_ap.offset, ap=[[0, 128], [1, C]]
                ),
            )
            # per-pair q/k biases (two heads per 128-partition tile)
            for t in range(2):
                bq_r = b_d["bq"][:].rearrange("(t p one) -> t p one", p=128, one=1)
                bk_r = b_d["bk"][:].rearrange("(t p one) -> t p one", p=128, one=1)
                nc.sync.dma_start(out=bqp[t], in_=bq_r[t])
                nc.sync.dma_start(out=bkp[t], in_=bk_r[t])
            # bp per o-chunk, gb = gamma * bp
            bp_r = b_d["bp"][:].rearrange("(t p one) -> t p one", p=128, one=1)

            # ---------- setup: load x, cast, weights transpose ----------
            with (
                tc.tile_pool(name="setup_sb", bufs=2) as sb,
                tc.tile_pool(name="setup_ps", bufs=2, space="PSUM") as sps,
            ):
                # keep the PE busy through the DMA-bound setup so the HAM
                # clock gate reaches (and keeps) full rate before the
                # projection matmuls start
                wps = sps.tile([128, 512], F32, tag="wps", name="wps")
                for _ in range(24):
                    nc.tensor.matmul(
                        wps, lhsT=wdum[:, 0:128], rhs=wdum, start=True, stop=True
                    )

                # weights: load natural [o, c], PE-transpose to [c, o] bf16
                wT_dst = {"wq": wqT, "wk": wkT, "wv": wvT}
                for name in ("wq", "wk", "wv", "wp"):
                    wn = [sb.tile([128, C], F32, tag=f"wnat{t}", name=f"wnat{t}") for t in range(2)]
                    w_r = w_d[name][:, :].rearrange("(t p) c -> t p c", p=128)
                    for t in range(2):
                        nc.sync.dma_start(out=wn[t], in_=w_r[t])
                    for i in range(2):  # o tile
                        for j in range(2):  # c tile
                            tp = sps.tile([128, 128], F32, tag="wtp", name="wtp")
                            nc.tensor.transpose(
                                tp, wn[i][:, j * 128 : (j + 1) * 128], ident
                            )
                            if name == "wp":
                                # split to per-head base-0 tiles via DMA
                                wp_st = sb.tile([128, 128], BF16, tag="wpst", name="wpst")
                                nc.vector.tensor_copy(out=wp_st, in_=tp)
                                for hh in range(2):
                                    h = 2 * j + hh
                                    nc.sync.dma_start(
                                        out=wpPh[h][0:64, i * 128 : (i + 1) * 128],
                                        in_=wp_st[64 * hh : 64 * hh + 64, :],
                                    )
                            else:
                                nc.vector.tensor_copy(
                                    out=wT_dst[name][j][:, i * 128 : (i + 1) * 128],
                                    in_=tp,
                                )

                for t in range(2):
                    xf = sb.tile([128, HW], F32, tag="xf", name="xf")
                    nc.sync.dma_start(out=xf, in_=x_t[t])
                    nc.scalar.copy(out=x16[t], in_=xf)
                for t in range(2):
                    nc.sync.dma_start(out=xb[t], in_=xq_t[t])
                    nc.vector.tensor_copy(out=xq16[t], in_=xb[t])
                    bp_t = sb.tile([128, 1], F32, tag="bpt", name="bpt")
                    nc.sync.dma_start(out=bp_t, in_=bp_r[t])
                    nc.vector.tensor_mul(out=gb[t], in0=bp_t, in1=gam)
                    # xb = xq + gamma*bp
                    nc.vector.tensor_scalar_add(out=xb[t], in0=xb[t], scalar1=gb[t])

            # ---------- K, Q, V projections ----------
            with tc.tile_pool(name="proj_ps", bufs=3, space="PSUM") as bps:
                for t in range(2):
                    for mb in range(HW // 512):
                        ps = bps.tile([128, 512], F32, tag="pk", name="pk")
                        for ci in range(2):
                            nc.tensor.matmul(
                                ps,
                                lhsT=wkT[ci][:, 128 * t : 128 * t + 128],
                                rhs=x16[ci][:, mb * 512 : (mb + 1) * 512],
                                start=(ci == 0),
                                stop=(ci == 1),
                            )
                        # write each head's half into its zero-padded kpad
                        # tile (same partitions); split across ScalarE/DVE so
                        # neither engine serializes the projection phase
                        for hh in range(2):
                            h = 2 * t + hh
                            sl = slice(64 * hh, 64 * hh + 64)
                            cols = slice(mb * 512, (mb + 1) * 512)
                            if hh == 0:
                                nc.scalar.activation(
                                    out=kpad[h][sl, cols],
                                    in_=ps[sl, :],
                                    func=mybir.ActivationFunctionType.Identity,
                                    bias=bkp[t][sl, :],
                                )
                            else:
                                nc.vector.tensor_scalar_add(
                                    out=kpad[h][sl, cols],
                                    in0=ps[sl, :],
                                    scalar1=bkp[t][sl, :],
                                )
                for t in range(2):
                    for nb in range(NHALF // 512):
                        ps = bps.tile([128, 512], F32, tag="pk", name="pk")
                        for ci in range(2):
                            nc.tensor.matmul(
                                ps,
                                lhsT=wqT[ci][:, 128 * t : 128 * t + 128],
                                rhs=xq16[ci][:, nb * 512 : (nb + 1) * 512],
                                start=(ci == 0),
                                stop=(ci == 1),
                            )
                        nc.scalar.activation(
                            out=qp[t][:, nb * 512 : (nb + 1) * 512],
                            in_=ps,
                            func=mybir.ActivationFunctionType.Identity,
                            bias=bqp[t],
                        )
                for mc in range(NCHUNK):
                    ps = bps.tile([128, C], F32, tag="pv", name="pv")
                    for ci in range(2):
                        nc.tensor.matmul(
                            ps,
                            lhsT=x16[ci][:, mc * 128 : (mc + 1) * 128],
                            rhs=wvT[ci][:, :],
                            start=(ci == 0),
                            stop=(ci == 1),
                        )
                    nc.vector.tensor_add(
                        out=vta[:, mc, :, 0:HD],
                        in0=ps.rearrange("p (h d) -> p h d", h=HEADS),
                        in1=bvb.rearrange("p (h d) -> p h d", h=HEADS),
                    )

            # ---------- attention ----------
            with (
                tc.tile_pool(name="st_ps", bufs=2, space="PSUM") as stp,
                tc.tile_pool(name="o_ps", bufs=2, space="PSUM") as op,
                tc.tile_pool(name="attn_sb", bufs=4) as asb,
            ):
                # normalize is emitted one group late so its reciprocal (DVE)
                # runs concurrently with the next group's matmul stream.
                def normalize_recip(ops):
                    # bf16 out is fine: the old path also cast fp32->bf16
                    # before the broadcast matmul
                    with nc.allow_low_precision(reason="recip broadcast was bf16 already"):
                        nc.vector.reciprocal(rbb[0:1, :], ops[HD : HD + 1, :])

                def normalize_apply(ops, h, n0):
                    bc = stp.tile([128, 1024], F32, tag="st", name="bc")
                    for half in range(2):
                        nc.tensor.matmul(
                            bc[:, half * 512 : (half + 1) * 512],
                            lhsT=onesP,
                            rhs=rbb[:, half * 512 : (half + 1) * 512],
                            start=True,
                            stop=True,
                        )
                    bcs = asb.tile([64, 1024], BF16, tag="bcs", name="bcs")
                    nc.vector.tensor_copy(out=bcs, in_=bc[0:64, :])
                    nc.vector.tensor_mul(
                        out=oh[h][0:HD, n0 : n0 + 1024], in0=ops[0:HD, :], in1=bcs
                    )

                def normalize(ops, h, n0):
                    normalize_recip(ops)
                    normalize_apply(ops, h, n0)

                pending = None
                for h in range(HEADS):
                    for nb2 in range(NHALF // 1024):
                        n0 = nb2 * 1024
                        ops = op.tile([128, 1024], F32, tag="ops", name="ops")
                        for mc in range(NCHUNK):
                            st = stp.tile([128, 1024], F32, tag="st", name="st")
                            for half in range(2):
                                nc.tensor.matmul(
                                    st[:, half * 512 : (half + 1) * 512],
                                    lhsT=kpad[h][:, mc * 128 : (mc + 1) * 128],
                                    rhs=qp[h // 2][
                                        :, n0 + half * 512 : n0 + (half + 1) * 512
                                    ],
                                    start=True,
                                    stop=True,
                                )
                            ex = asb.tile([128, 1024], BF16, tag="ex", name="ex")
                            nc.scalar.activation(
                                out=ex,
                                in_=st,
                                func=mybir.ActivationFunctionType.Exp,
                                scale=float(SCALE),
                            )
                            for half in range(2):
                                nc.tensor.matmul(
                                    ops[:, half * 512 : (half + 1) * 512],
                                    lhsT=vta[:, mc, h, :],
                                    rhs=ex[:, half * 512 : (half + 1) * 512],
                                    start=(mc == 0),
                                    stop=(mc == NCHUNK - 1),
                                )
                            # previous group's normalize, emitted mid-stream:
                            # by now its reciprocal (started at this group's
                            # first matmul) is done, so the broadcast matmul
                            # slots into the PE stream without a stall
                            if mc == 8 and pending is not None:
                                normalize(*pending)
                                pending = None
                        pending = (ops, h, n0)

                # ---------- output projection + residual ----------
                # PSUM tiles come from the stp pool (same shape/tag as st, so
                # the final group's ops accumulators in the op pool are never
                # recycled before their normalize reads them). The final
                # normalize is split: its reciprocal (DVE, ~6.5us) is emitted
                # first so it runs under the first two projection tiles'
                # matmuls; the broadcast+multiply lands before the np2=1
                # tiles, whose head-3 matmuls are the only consumers.
                normalize_recip(pending[0])
                for np2 in range(2):  # query-column pair (np2*1024)
                    for oc in range(2):  # output-channel tile
                        if np2 == 1 and oc == 0 and pending is not None:
                            normalize_apply(*pending)
                            pending = None
                        pst = stp.tile([128, 1024], F32, tag="st", name="pst")
                        for h in range(HEADS):
                            for half in range(2):
                                c0 = np2 * 1024 + half * 512
                                nc.tensor.matmul(
                                    pst[:, half * 512 : (half + 1) * 512],
                                    lhsT=wpPh[h][:, oc * 128 : (oc + 1) * 128],
                                    rhs=oh[h][:, c0 : c0 + 512],
                                    start=(h == 0),
                                    stop=(h == HEADS - 1),
                                )
                        res = asb.tile([128, 1024], F32, tag="res", name="res")
                        nc.vector.scalar_tensor_tensor(
                            out=res,
                            in0=pst,
                            scalar=gam[:, 0:1],
                            in1=xb[oc][:, np2 * 1024 : (np2 + 1) * 1024],
                            op0=mybir.AluOpType.mult,
                            op1=mybir.AluOpType.add,
                        )
                        nc.sync.dma_start(
                            out=out_t[oc, :, np2 * 1024 : (np2 + 1) * 1024], in_=res
                        )

    if fix:
        _fix_tail_drain(nc)
    return nc


_NC_CACHE = None


def _get_nc():
    global _NC_CACHE
    if _NC_CACHE is None:
        _NC_CACHE = build()
    return _NC_CACHE


def kernel(x, wq, bq, wk, bk, wv, bv, wp, bp, gamma):
    from concourse.bass_utils import run_bass_kernel_spmd

    nc = _get_nc()
    x = np.ascontiguousarray(np.asarray(x, np.float32)).reshape(B, C, HW)
    common = {
        "wq": np.ascontiguousarray(np.asarray(wq, np.float32)),
        "wk": np.ascontiguousarray(np.asarray(wk, np.float32)),
        "wv": np.ascontiguousarray(np.asarray(wv, np.float32)),
        "wp": np.ascontiguousarray(np.asarray(wp, np.float32)),
        "bq": np.ascontiguousarray(np.asarray(bq, np.float32)),
        "bk": np.ascontiguousarray(np.asarray(bk, np.float32)),
        "bv": np.ascontiguousarray(np.asarray(bv, np.float32)),
        "bp": np.ascontiguousarray(np.asarray(bp, np.float32)),
        "gamma": np.ascontiguousarray(np.asarray(gamma, np.float32)),
    }
    in_maps = []
    for core in range(8):
        b, j = core // 2, core % 2
        m = dict(common)
        m["x"] = np.ascontiguousarray(x[b])
        m["xq"] = np.ascontiguousarray(x[b][:, j * NHALF : (j + 1) * NHALF])
        in_maps.append(m)

    res = run_bass_kernel_spmd(nc, in_maps, core_ids=list(range(8)), trace=False)
    out = np.empty((B, C, HW), np.float32)
    for core in range(8):
        b, j = core // 2, core % 2
        out[b][:, j * NHALF : (j + 1) * NHALF] = res.results[core]["out"]
    return out.reshape(B, C, H, W)


# revision 45
# speedup vs baseline: 1.1789x; 1.1789x over previous
"""Trainium2 Bass kernel for MultiHeadSelfAttention2D.

Problem: x(4,256,64,64); q,k,v,proj 1x1-conv projections; 4 heads x 64 dim;
full 4096x4096 attention per (batch,head); out = gamma*proj + x.

Sharding: 8 cores = batch(4) x query-half(2). Each core computes its full
output slice out[b][:, nhalf] on-device:
  - K,V projected from full x[b]; Q from its query half only.
  - Flash-style attention: S^T = K^T Q computed per 128-key chunk into PSUM,
    exp on ScalarE (scores are bounded for N(0,1)-scaled inputs, so no
    max-subtraction pass), then PV matmul with a ones-column appended to V^T
    so the softmax denominator accumulates in the same matmul.
  - Normalization via rank-1 (ones x recip) matmul broadcast + one multiply.
  - Output projection + bias + gamma + residual on-device.
Host only concatenates the 8 slices.

v2: every matmul is forced into the uniform (128,128) PE tile mode.  The
v1 kernel alternated 64-contract S^T matmuls (tile mode 64x128) with
128-contract PV matmuls (128x128); each switch drains the systolic array
and the PE clock stays at the cold 1.2 GHz rate (measured 426-570 ns per
N=512 matmul vs 216 ns warm).  Contract dims below 128 are zero-padded:
  - kpad[h]: [128, HW] K tile per head, head data on its own 64-partition
    half, zeros on the other half (matches qp's packed-head-pair rows).
  - wpPh[h]/oh[h]: output projection runs 4 accumulating 128-contract
    matmuls with the upper 64 rows zeroed.
  - normalize broadcast: ones-row lhsT padded to [128,128] (partition 0 is
    ones, the rest zeros) so the rank-1 broadcast is also (128,128).
"""

import numpy as np

import concourse.bass as bass
import concourse.mybir as mybir
import concourse.tile as tile

B, C, H, W, HEADS = 4, 256, 64, 64, 4
HD = C // HEADS  # 64
HW = H * W  # 4096
NHALF = HW // 2  # 2048
NCHUNK = HW // 128  # 32 key chunks
SCALE = 1.0 / np.sqrt(HD)
F32 = mybir.dt.float32
BF16 = mybir.dt.bfloat16


def _fix_tail_drain(nc, keep=1):
    """This walrus build rejects instructions with more than a couple of
    semaphore waits. Inserting a same-engine NoOp immediately before an
    instruction is semantically identical (the engine blocks at the NoOp
    instead), so split any excess waits onto adjacent NoOps."""
    fn = nc.m.functions[0]
    for bi, blk in enumerate(fn.blocks):
        insts = list(blk.instructions)
        changed = False
        new_list = []
        for ins in insts:
            si = ins.sync_info
            if si is not None and len(si.on_wait) > keep:
                waits = list(si.on_wait)
                kept, excess = waits[:keep], waits[keep:]
                for j, w in enumerate(excess):
                    new_list.append(
                        mybir.InstNoOp(
                            name=f"waitfix-{bi}-{ins.name}-{j}",
                            engine=ins.engine,
                            sync_info=mybir.SyncInfo(on_wait=[w], on_update=[]),
                        )
                    )
                ins.sync_info = mybir.SyncInfo(on_wait=kept, on_update=si.on_update)
                changed = True
            new_list.append(ins)
        if changed:
            blk.instructions = new_list


def build(fix=True):
    from concourse.masks import make_identity

    nc = bass.Bass("TRN2", target_bir_lowering=False)

    # x arrives host-cast to bf16 (saves half the input DMA; the bf16
    # residual costs ~2e-3 l2, well inside tolerance) and host-rotated so
    # this core's query half occupies columns 0:NHALF. Attention output for
    # a query is invariant to key order, so K/V may use the rotated layout.
    x_d = nc.dram_tensor("x", [C, HW], BF16, kind="ExternalInput")
    w_d = {
        n: nc.dram_tensor(n, [C, C], F32, kind="ExternalInput")
        for n in ("wq", "wk", "wv", "wp")
    }
    b_d = {
        n: nc.dram_tensor(n, [C], F32, kind="ExternalInput")
        for n in ("bq", "bk", "bv", "bp")
    }
    gamma_d = nc.dram_tensor("gamma", [1], F32, kind="ExternalInput")
    out_d = nc.dram_tensor("out", [C, NHALF], F32, kind="ExternalOutput")

    x_t = x_d[:, :].rearrange("(t p) m -> t p m", p=128)
    out_t = out_d[:, :].rearrange("(t p) n -> t p n", p=128)

    with tile.TileContext(nc) as tc:
        with tc.tile_pool(name="persist", bufs=1) as pp:
            # ---------- persistent tiles ----------
            x16 = [pp.tile([128, HW], BF16, tag=f"x16_{t}", name=f"x16_{t}") for t in range(2)]
            xb = [pp.tile([128, NHALF], BF16, tag=f"xb_{t}", name=f"xb_{t}") for t in range(2)]
            # per-head K, zero-padded to the full 128-partition contract
            kpad = [pp.tile([128, HW], BF16, tag=f"kpad_{h}", name=f"kpad_{h}") for h in range(HEADS)]
            qp = [pp.tile([128, NHALF], BF16, tag=f"qp_{t}", name=f"qp_{t}") for t in range(2)]
            # per-head attention output, rows 64-127 zero (128-contract out proj)
            oh = [pp.tile([128, NHALF], BF16, tag=f"oh_{h}", name=f"oh_{h}") for h in range(HEADS)]
            # PV lhsT reads 128 weight columns so NumWeights==128 keeps Fast
            # Weight Load enabled (a 65-column lhsT serializes LDWEIGHTS with
            # the matmul, +160ns per PV matmul). Heads are packed at stride
            # 65 ([v(64) | ones(1)] each) and the 128-column read simply
            # overlaps the next head's data: the overlap lands in output
            # partitions 65-127, which are never read. Only the 63-column
            # tail after head 3 needs zeroing (finite filler).
            vta = pp.tile([128, NCHUNK, HEADS * 65 + 63], BF16, tag="vta", name="vta")
            wqT = [pp.tile([128, C], BF16, tag=f"wqT_{t}", name=f"wqT_{t}") for t in range(2)]
            wkT = [pp.tile([128, C], BF16, tag=f"wkT_{t}", name=f"wkT_{t}") for t in range(2)]
            wvT = [pp.tile([128, C], BF16, tag=f"wvT_{t}", name=f"wvT_{t}") for t in range(2)]
            # per-head wp^T slice on rows 0-63, rows 64-127 zero
            wpPh = [pp.tile([128, C], BF16, tag=f"wpPh_{h}", name=f"wpPh_{h}") for h in range(HEADS)]
            bqp = [pp.tile([128, 1], F32, tag=f"bqp_{t}", name=f"bqp_{t}") for t in range(2)]
            bkp = [pp.tile([128, 1], F32, tag=f"bkp_{t}", name=f"bkp_{t}") for t in range(2)]
            bvb = pp.tile([128, C], F32, tag="bvb", name="bvb")
            gam = pp.tile([128, 1], F32, tag="gam", name="gam")
            gb = [pp.tile([128, 1], F32, tag=f"gb_{t}", name=f"gb_{t}") for t in range(2)]
            ident = pp.tile([128, 128], F32, tag="ident", name="ident")
            wdum = pp.tile([128, 512], BF16, tag="wdum", name="wdum")
            # rank-1 softmax-denominator broadcast, padded to (128,128) mode:
            # onesP partition 0 is all-ones, partitions 1-127 zero; rbb row 0
            # carries 1/denominator, rows 1-127 stay zero.
            onesP = pp.tile([128, 128], BF16, tag="onesP", name="onesP")
            rbb = pp.tile([128, 1024], BF16, tag="rbb", name="rbb")

            # identity first: the weight transposes (PE) wait on it, and
            # anything queued ahead of it on its engine delays the whole
            # weight pipeline
            make_identity(nc, ident)
            nc.vector.memset(wdum, 0.0)
            # kpad zero-halves on GpSimd (otherwise idle), in K-projection
            # consumption order; everything else is zero-filled after the
            # weight section so the DVE queue stays clear for the
            # transpose-copy pipeline
            for h in range(HEADS):
                sl = slice(64 * (1 - (h % 2)), 64 * (1 - (h % 2)) + 64)
                nc.gpsimd.memset(kpad[h][sl, :], 0.0)

            # x DMA first: it is the critical input (the projections wait on
            # it) and each dma_start costs ~600ns of issue time on the Sync
            # queue, so anything queued ahead delays the transfer start
            for t in range(2):
                nc.sync.dma_start(out=x16[t], in_=x_t[t])

            # gamma broadcast to all partitions
            g_ap = gamma_d[:]
            nc.sync.dma_start(
                out=gam,
                in_=bass.AP(tensor=g_ap.tensor, offset=g_ap.offset, ap=[[0, 128], [1, 1]]),
            )
            # bv broadcast [128, C]
            bv_ap = b_d["bv"][:]
            nc.sync.dma_start(
                out=bvb,
                in_=bass.AP(
                    tensor=bv_ap.tensor, offset=bv_ap.offset, ap=[[0, 128], [1, C]]
                ),
            )
            # per-pair q/k biases (two heads per 128-partition tile)
            for t in range(2):
                bq_r = b_d["bq"][:].rearrange("(t p one) -> t p one", p=128, one=1)
                bk_r = b_d["bk"][:].rearrange("(t p one) -> t p one", p=128, one=1)
                nc.sync.dma_start(out=bqp[t], in_=bq_r[t])
                nc.sync.dma_start(out=bkp[t], in_=bk_r[t])
            # bp per o-chunk, gb = gamma * bp
            bp_r = b_d["bp"][:].rearrange("(t p one) -> t p one", p=128, one=1)

            # ---------- setup: load x, cast, weights transpose ----------
            with (
                tc.tile_pool(name="setup_sb", bufs=2) as sb,
                tc.tile_pool(name="setup_ps", bufs=2, space="PSUM") as sps,
            ):
                # keep the PE busy through the DMA-bound setup so the HAM
                # clock gate reaches (and keeps) full rate before the
                # projection matmuls start
                wps = sps.tile([128, 512], F32, tag="wps", name="wps")
                for _ in range(24):
                    nc.tensor.matmul(
                        wps, lhsT=wdum[:, 0:128], rhs=wdum, start=True, stop=True
                    )

                # weights: load natural [o, c], PE-transpose to [c, o] bf16.
                # Distinct tile tags per weight so no DMA waits on a
                # transpose of the previous weight to free its slot.
                wT_dst = {"wq": wqT, "wk": wkT, "wv": wvT}
                for name in ("wq", "wk", "wv", "wp"):
                    wn = [
                        sb.tile([128, C], F32, tag=f"wnat_{name}{t}", name=f"wnat_{name}{t}")
                        for t in range(2)
                    ]
                    w_r = w_d[name][:, :].rearrange("(t p) c -> t p c", p=128)
                    for t in range(2):
                        nc.sync.dma_start(out=wn[t], in_=w_r[t])
                    for i in range(2):  # o tile
                        for j in range(2):  # c tile
                            tp = sps.tile([128, 128], F32, tag="wtp", name="wtp")
                            nc.tensor.transpose(
                                tp, wn[i][:, j * 128 : (j + 1) * 128], ident
                            )
                            if name == "wp":
                                # split to per-head base-0 tiles via DMA
                                wp_st = sb.tile([128, 128], BF16, tag="wpst", name="wpst")
                                nc.vector.tensor_copy(out=wp_st, in_=tp)
                                for hh in range(2):
                                    h = 2 * j + hh
                                    nc.sync.dma_start(
                                        out=wpPh[h][0:64, i * 128 : (i + 1) * 128],
                                        in_=wp_st[64 * hh : 64 * hh + 64, :],
                                    )
                            else:
                                nc.vector.tensor_copy(
                                    out=wT_dst[name][j][:, i * 128 : (i + 1) * 128],
                                    in_=tp,
                                )

                for t in range(2):
                    bp_t = sb.tile([128, 1], F32, tag="bpt", name="bpt")
                    nc.sync.dma_start(out=bp_t, in_=bp_r[t])
                    nc.vector.tensor_mul(out=gb[t], in0=bp_t, in1=gam)
                    # xb = x_own + gamma*bp (the core's query half is
                    # columns 0:NHALF of the rotated x)
                    nc.vector.tensor_scalar_add(
                        out=xb[t], in0=x16[t][:, 0:NHALF], scalar1=gb[t]
                    )

                # vta zero-tail + ones columns, before the V-projection
                # writes into it (DVE program order)
                nc.vector.memset(vta[:, :, HEADS * 65 : HEADS * 65 + 63], 0.0)
                for h in range(HEADS):
                    nc.vector.memset(vta[:, :, 65 * h + HD : 65 * h + HD + 1], 1.0)

            # ---------- K, Q, V projections ----------
            with tc.tile_pool(name="proj_ps", bufs=3, space="PSUM") as bps:
                for t in range(2):
                    for mb in range(HW // 512):
                        ps = bps.tile([128, 512], F32, tag="pk", name="pk")
                        for ci in range(2):
                            nc.tensor.matmul(
                                ps,
                                lhsT=wkT[ci][:, 128 * t : 128 * t + 128],
                                rhs=x16[ci][:, mb * 512 : (mb + 1) * 512],
                                start=(ci == 0),
                                stop=(ci == 1),
                            )
                        # write each head's half into its zero-padded kpad
                        # tile (same partitions); split across ScalarE/DVE so
                        # neither engine serializes the projection phase
                        for hh in range(2):
                            h = 2 * t + hh
                            sl = slice(64 * hh, 64 * hh + 64)
                            cols = slice(mb * 512, (mb + 1) * 512)
                            if hh == 0:
                                nc.scalar.activation(
                                    out=kpad[h][sl, cols],
                                    in_=ps[sl, :],
                                    func=mybir.ActivationFunctionType.Identity,
                                    bias=bkp[t][sl, :],
                                )
                            else:
                                nc.vector.tensor_scalar_add(
                                    out=kpad[h][sl, cols],
                                    in0=ps[sl, :],
                                    scalar1=bkp[t][sl, :],
                                )
                for t in range(2):
                    for nb in range(NHALF // 512):
                        ps = bps.tile([128, 512], F32, tag="pk", name="pk")
                        for ci in range(2):
                            nc.tensor.matmul(
                                ps,
                                lhsT=wqT[ci][:, 128 * t : 128 * t + 128],
                                rhs=x16[ci][:, nb * 512 : (nb + 1) * 512],
                                start=(ci == 0),
                                stop=(ci == 1),
                            )
                        nc.scalar.activation(
                            out=qp[t][:, nb * 512 : (nb + 1) * 512],
                            in_=ps,
                            func=mybir.ActivationFunctionType.Identity,
                            bias=bqp[t],
                        )
                for mc in range(NCHUNK):
                    ps = bps.tile([128, C], F32, tag="pv", name="pv")
                    for ci in range(2):
                        nc.tensor.matmul(
                            ps,
                            lhsT=x16[ci][:, mc * 128 : (mc + 1) * 128],
                            rhs=wvT[ci][:, :],
                            start=(ci == 0),
                            stop=(ci == 1),
                        )
                    nc.vector.tensor_add(
                        out=vta[:, mc, 0 : HEADS * 65].rearrange(
                            "p (h e) -> p h e", h=HEADS
                        )[:, :, 0:HD],
                        in0=ps.rearrange("p (h d) -> p h d", h=HEADS),
                        in1=bvb.rearrange("p (h d) -> p h d", h=HEADS),
                    )

                # remaining zero-fills; their consumers (normalize, output
                # projection) run much later, so they go after the
                # projection-phase DVE work
                nc.vector.memset(onesP, 0.0)
                nc.vector.memset(onesP[0:1, :], 1.0)
                nc.vector.memset(rbb, 0.0)
                for h in range(HEADS):
                    nc.vector.memset(oh[h][64:128, :], 0.0)
                    nc.vector.memset(wpPh[h][64:128, :], 0.0)

            # ---------- attention ----------
            with (
                tc.tile_pool(name="st_ps", bufs=2, space="PSUM") as stp,
                tc.tile_pool(name="o_ps", bufs=2, space="PSUM") as op,
                tc.tile_pool(name="attn_sb", bufs=4) as asb,
            ):
                # normalize is emitted one group late so its reciprocal (DVE)
                # runs concurrently with the next group's matmul stream.
                def normalize_recip(ops):
                    # DVE reciprocal is ~8 cycles *per element per
                    # partition*; on the [1,1024] denominator row that is a
                    # 6.5us serial op. Instead: ScalarE copies the PSUM row
                    # to SBUF (PSUM is not DMA-able), a DMA reshapes it to
                    # [64,16] so 64 lanes divide in parallel (~0.1us), and a
                    # second DMA lands the recips back in rbb row 0.
                    d1 = asb.tile([1, 1024], BF16, tag="d1", name="d1")
                    nc.scalar.copy(out=d1, in_=ops[HD : HD + 1, :])
                    t64 = asb.tile([64, 16], BF16, tag="t64", name="t64")
                    nc.sync.dma_start(out=t64, in_=d1)
                    rec = asb.tile([64, 16], BF16, tag="rec64", name="rec64")
                    with nc.allow_low_precision(reason="recip broadcast is bf16 anyway"):
                        nc.vector.reciprocal(rec, t64)
                    nc.sync.dma_start(out=rbb[0:1, :], in_=rec)

                def normalize_apply(ops, h, n0):
                    bc = stp.tile([128, 1024], F32, tag="st", name="bc")
                    for half in range(2):
                        nc.tensor.matmul(
                            bc[:, half * 512 : (half + 1) * 512],
                            lhsT=onesP,
                            rhs=rbb[:, half * 512 : (half + 1) * 512],
                            start=True,
                            stop=True,
                        )
                    bcs = asb.tile([64, 1024], BF16, tag="bcs", name="bcs")
                    # ScalarE does this PSUM->SBUF cast: the DVE is the
                    # busier engine in the attention steady state
                    nc.scalar.copy(out=bcs, in_=bc[0:64, :])
                    nc.vector.tensor_mul(
                        out=oh[h][0:HD, n0 : n0 + 1024], in0=ops[0:HD, :], in1=bcs
                    )

                def normalize(ops, h, n0):
                    normalize_recip(ops)
                    normalize_apply(ops, h, n0)

                # Schraudolph bf16 exp for the DVE path: bf16 bits of 2^t are
                # (t+127)*128 with a piecewise-linear mantissa, so
                #   bits_i16 = round(score * SCALE*log2(e)*128 + (16256 - C))
                # bit-cast back as bf16 gives exp(score*SCALE) with a +-3%
                # ripple; C centers the approximation so its mean matches the
                # exact ScalarE exp (the two paths share one softmax sum).
                K_SCH = float(SCALE * np.log2(np.e) * 128.0)
                B_SCH = float(127.0 * 128.0 - 7.37)

                # One flat pipeline over all (head, query-block, key-chunk)
                # units. The PV matmuls of unit i are emitted after unit
                # i+2's S^T matmuls: by the time PV issues, its exp (which
                # starts only after unit i's S^T completes and takes
                # ~1.1-1.2us on either engine) has finished, so PV streams
                # back-to-back (216 ns) instead of stalling the PE.
                # exp alternates ScalarE (exact) / DVE (Schraudolph) so the
                # two engines halve the softmax work between them.
                pending = None
                pvq = []  # [(ops, h, mc, ex)] units awaiting their PV
                ops = None

                def emit_pv(pops, ph, pmc, pex):
                    for half in range(2):
                        nc.tensor.matmul(
                            pops[:, half * 512 : (half + 1) * 512],
                            lhsT=vta[:, pmc, 65 * ph : 65 * ph + 128],
                            rhs=pex[:, half * 512 : (half + 1) * 512],
                            start=(pmc == 0),
                            stop=(pmc == NCHUNK - 1),
                        )

                for ui, (h, nb2, mc) in enumerate(
                    (h, nb2, mc)
                    for h in range(HEADS)
                    for nb2 in range(NHALF // 1024)
                    for mc in range(NCHUNK)
                ):
                    n0 = nb2 * 1024
                    if mc == 0:
                        ops = op.tile([128, 1024], F32, tag="ops", name="ops")
                    st = stp.tile([128, 1024], F32, tag="st", name="st")
                    for half in range(2):
                        nc.tensor.matmul(
                            st[:, half * 512 : (half + 1) * 512],
                            lhsT=kpad[h][:, mc * 128 : (mc + 1) * 128],
                            rhs=qp[h // 2][
                                :, n0 + half * 512 : n0 + (half + 1) * 512
                            ],
                            start=True,
                            stop=True,
                        )
                    ex = asb.tile([128, 1024], BF16, tag="ex", name="ex")
                    if ui % 8 not in (1, 3, 5):
                        nc.scalar.activation(
                            out=ex,
                            in_=st,
                            func=mybir.ActivationFunctionType.Exp,
                            scale=float(SCALE),
                        )
                    else:
                        nc.vector.tensor_scalar(
                            out=ex[:, :].bitcast(mybir.dt.int16),
                            in0=st,
                            scalar1=K_SCH,
                            scalar2=B_SCH,
                            op0=mybir.AluOpType.mult,
                            op1=mybir.AluOpType.add,
                        )
                    pvq.append((ops, h, mc, ex))
                    if len(pvq) > 3:
                        emit_pv(*pvq.pop(0))
                    # previous group's normalize, emitted mid-stream: its
                    # reciprocal (started at this group's first matmul) is
                    # done by now, so the broadcast matmul slots into the PE
                    # stream without a stall
                    if mc == 8 and pending is not None:
                        normalize(*pending)
                        pending = None
                    if mc == NCHUNK - 1:
                        pending = (ops, h, n0)
                for p in pvq:
                    emit_pv(*p)

                # ---------- output projection + residual ----------
                # PSUM tiles come from the stp pool (same shape/tag as st, so
                # the final group's ops accumulators in the op pool are never
                # recycled before their normalize reads them). The final
                # normalize is split: its reciprocal (DVE, ~6.5us) is emitted
                # first so it runs under the first two projection tiles'
                # matmuls; the broadcast+multiply lands before the np2=1
                # tiles, whose head-3 matmuls are the only consumers.
                normalize_recip(pending[0])
                for np2 in range(2):  # query-column pair (np2*1024)
                    for oc in range(2):  # output-channel tile
                        if np2 == 1 and oc == 0 and pending is not None:
                            normalize_apply(*pending)
                            pending = None
                        pst = stp.tile([128, 1024], F32, tag="st", name="pst")
                        for h in range(HEADS):
                            for half in range(2):
                                c0 = np2 * 1024 + half * 512
                                nc.tensor.matmul(
                                    pst[:, half * 512 : (half + 1) * 512],
                                    lhsT=wpPh[h][:, oc * 128 : (oc + 1) * 128],
                                    rhs=oh[h][:, c0 : c0 + 512],
                                    start=(h == 0),
                                    stop=(h == HEADS - 1),
                                )
                        res = asb.tile([128, 1024], F32, tag="res", name="res")
                        nc.vector.scalar_tensor_tensor(
                            out=res,
                            in0=pst,
                            scalar=gam[:, 0:1],
                            in1=xb[oc][:, np2 * 1024 : (np2 + 1) * 1024],
                            op0=mybir.AluOpType.mult,
                            op1=mybir.AluOpType.add,
                        )
                        nc.sync.dma_start(
                            out=out_t[oc, :, np2 * 1024 : (np2 + 1) * 1024], in_=res
                        )

    if fix:
        _fix_tail_drain(nc)
    return nc


_NC_CACHE = None


def _get_nc():
    global _NC_CACHE
    if _NC_CACHE is None:
        _NC_CACHE = build()
    return _NC_CACHE


def make_in_maps(x, wq, bq, wk, bk, wv, bv, wp, bp, gamma):
    import ml_dtypes

    x = np.ascontiguousarray(np.asarray(x, np.float32)).reshape(B, C, HW)
    x16 = x.astype(ml_dtypes.bfloat16)
    common = {
        "wq": np.ascontiguousarray(np.asarray(wq, np.float32)),
        "wk": np.ascontiguousarray(np.asarray(wk, np.float32)),
        "wv": np.ascontiguousarray(np.asarray(wv, np.float32)),
        "wp": np.ascontiguousarray(np.asarray(wp, np.float32)),
        "bq": np.ascontiguousarray(np.asarray(bq, np.float32)),
        "bk": np.ascontiguousarray(np.asarray(bk, np.float32)),
        "bv": np.ascontiguousarray(np.asarray(bv, np.float32)),
        "bp": np.ascontiguousarray(np.asarray(bp, np.float32)),
        "gamma": np.ascontiguousarray(np.asarray(gamma, np.float32)),
    }
    in_maps = []
    for core in range(8):
        b, j = core // 2, core % 2
        m = dict(common)
        # rotate so this core's query half is first; key order is
        # irrelevant to the attention output
        if j == 0:
            m["x"] = np.ascontiguousarray(x16[b])
        else:
            m["x"] = np.ascontiguousarray(
                np.concatenate([x16[b][:, NHALF:], x16[b][:, :NHALF]], axis=1)
            )
        in_maps.append(m)
    return in_maps


def kernel(x, wq, bq, wk, bk, wv, bv, wp, bp, gamma):
    from concourse.bass_utils import run_bass_kernel_spmd

    nc = _get_nc()
    in_maps = make_in_maps(x, wq, bq, wk, bk, wv, bv, wp, bp, gamma)

    res = run_bass_kernel_spmd(nc, in_maps, core_ids=list(range(8)), trace=False)
    out = np.empty((B, C, HW), np.float32)
    for core in range(8):
        b, j = core // 2, core % 2
        out[b][:, j * NHALF : (j + 1) * NHALF] = res.results[core]["out"]
    return out.reshape(B, C, H, W)
